# revision 2
# baseline (speedup 1.0000x reference)
"""Trainium2 Bass kernel for nn_Diag: out[n, d] = input[n, d] * W[d].

Full input [200000, 512] f32 is sharded row-wise (data parallel) across 8
NeuronCores; W [512] is replicated. Per core: [25000, 512].

Per-core layout: view each 1920-row block as [128 partitions x (15 rows * 512)]
so every DMA moves 30 KB contiguous per partition (3.75 MiB per transfer).
W is broadcast to all 128 partitions once and replicated 15x along the free
dim so each block needs a single DVE tensor_mul. Loads issue on the SyncE
HWDGE ring and stores on the ScalarE HWDGE ring so the two directions stream
concurrently. 40 leftover rows go through a small [40, 512] tile.
"""

import dataclasses

import numpy as np

N_CORES = 8
N_NODES = 200000
D = 512
ROWS_PER_CORE = N_NODES // N_CORES  # 25000
R = 15  # DRAM rows packed into each SBUF partition per tile
TILE_ROWS = 128 * R  # 1920
NT = ROWS_PER_CORE // TILE_ROWS  # 13 full tiles
REM = ROWS_PER_CORE - NT * TILE_ROWS  # 40 leftover rows
BUFS = 4

_NC_CACHE = {}


def _build_nc(repeat=1):
    """Build the per-core program. `repeat` > 1 emits the full pass that many
    times back-to-back inside one NEFF (used only for wall-clock benchmarking;
    pool-slot reuse serializes iterations into one continuous tile stream)."""
    import concourse.tile as tile
    from concourse import bacc, mybir

    nc = bacc.Bacc(
        "TRN2", target_bir_lowering=False, debug=False, enable_asserts=False
    )
    f32 = mybir.dt.float32
    x = nc.dram_tensor("x", [ROWS_PER_CORE, D], f32, kind="ExternalInput").ap()
    w = nc.dram_tensor("w", [D], f32, kind="ExternalInput").ap()
    y = nc.dram_tensor("y", [ROWS_PER_CORE, D], f32, kind="ExternalOutput").ap()

    with tile.TileContext(nc) as tc:
        with (
            tc.tile_pool(name="wpool", bufs=1) as wpool,
            tc.tile_pool(name="data", bufs=BUFS) as data,
        ):
            wt = wpool.tile([128, D], f32)
            nc.sync.dma_start(wt[0:1, :], w[None, :])
            nc.gpsimd.partition_broadcast(wt[:], wt[0:1, :])
            # Replicate W R times along the free dim with a stride-0 read AP
            # so each big tile needs one full-width tensor_mul.
            wrep = wpool.tile([128, R * D], f32)
            src_rep = dataclasses.replace(
                wt[:, :], ap=[wt[:, :].ap[0], [0, R], wt[:, :].ap[1]]
            )
            nc.vector.tensor_copy(wrep[:].rearrange("p (r d) -> p r d", r=R), src_rep)

            for _ in range(repeat):
                for t in range(NT):
                    dtile = data.tile([128, R * D], f32, tag="dtile")
                    nc.sync.dma_start(
                        dtile[:],
                        x[t * TILE_ROWS : (t + 1) * TILE_ROWS, :].rearrange(
                            "(p r) d -> p (r d)", p=128
                        ),
                    )
                    nc.vector.tensor_mul(dtile[:], dtile[:], wrep[:])
                    nc.scalar.dma_start(
                        y[t * TILE_ROWS : (t + 1) * TILE_ROWS, :].rearrange(
                            "(p r) d -> p (r d)", p=128
                        ),
                        dtile[:],
                    )
                if REM:
                    rt = data.tile([128, D], f32, tag="rem")
                    nc.sync.dma_start(rt[0:REM, :], x[NT * TILE_ROWS :, :])
                    nc.vector.tensor_mul(rt[0:REM, :], rt[0:REM, :], wt[0:REM, :])
                    nc.scalar.dma_start(y[NT * TILE_ROWS :, :], rt[0:REM, :])
    nc.compile()
    return nc


def _run(input, W, trace=False, repeat=1, **kw):
    """Shard, execute on 8 cores, gather. Returns (full_output, BassKernelResults)."""
    from concourse import bass_utils

    if repeat not in _NC_CACHE:
        _NC_CACHE[repeat] = _build_nc(repeat)
    nc = _NC_CACHE[repeat]

    inp = np.ascontiguousarray(np.asarray(input), dtype=np.float32)
    Wf = np.ascontiguousarray(np.asarray(W), dtype=np.float32)
    shards = np.split(inp, N_CORES, axis=0)
    in_maps = [{"x": s, "w": Wf} for s in shards]
    res = bass_utils.run_bass_kernel_spmd(
        nc, in_maps, core_ids=list(range(N_CORES)), trace=trace, **kw
    )
    out = np.concatenate([r["y"] for r in res.results], axis=0)
    return out, res


def kernel(input, A, W):
    out, _ = _run(input, W)
    return out
